# revision 59
# baseline (speedup 1.0000x reference)
"""BERT-with-RoPE attention layer on 8 Trainium2 NeuronCores.

Sharding: core c handles (batch b = c//2, sequence-half hf = c%2).
Each core computes k/v for its batch's full 2048 rows (k/v work duplicated
2x across the pair) and q + attention + out-projection for its own 1024
query rows, so the 8 output shards are disjoint and the host gather is a
pure concatenation (no collectives).

v6 — pair-pipelined: the q/k projection (+RoPE) for pair p+2 is emitted
inside pair p's attention block stream, so the PE never idles while the
scalar engine grinds through the exp stream. Phase order on device:
  B:  v = xT^T @ Wv (SBUF-resident, ones column via memset)
  loop over pairs p (pair-major, hf inner):
      attention(p, hf=0/1) with qk-projection spans for pair p+2
      interleaved between score/ctx blocks; softmax via ones-column sums,
      normalize epilogue off the PE critical path
  D:  outT = Wout^T-slices @ ctxT (weights preloaded, PSUM borrowed from
      the scores pool; runs under the tail of the exp stream)
All q/k/sw/scores/D PSUM accumulators share one [128,2,512] tile pool
(6 banks) + 2 banks for the ctx accumulators = exactly 8 PSUM banks.
"""

import os
import numpy as np

B, S, H = 4, 2048, 1024
NH, DH = 16, 64
HALF = DH // 2
SQ = S // 2  # query rows per core
KC = H // 128  # hidden contraction chunks
ROPE_BASE = 10000.0
N_CORES = 8

_nc_cache = None
last_results = None


def _build_nc():
    import concourse.bacc as bacc
    import concourse.mybir as mybir
    from concourse.tile import TileContext

    f32 = mybir.dt.float32
    bf16 = mybir.dt.bfloat16
    Exp = mybir.ActivationFunctionType.Exp
    Ident = mybir.ActivationFunctionType.Identity
    Copy = mybir.ActivationFunctionType.Copy
    MUL = mybir.AluOpType.mult
    ADD = mybir.AluOpType.add

    nc = bacc.Bacc(None, target_bir_lowering=False)

    xT_d = nc.dram_tensor("xT", [KC, 128, S], bf16, kind="ExternalInput")
    pswap_d = nc.dram_tensor("pswap", [128, 128], bf16, kind="ExternalInput")
    wqk_d = nc.dram_tensor("wqk", [16, 128, KC, 128], bf16, kind="ExternalInput")
    wv_d = nc.dram_tensor("wv", [128, KC, H], bf16, kind="ExternalInput")
    wout_d = nc.dram_tensor("wout", [128, 8, KC, 128], bf16, kind="ExternalInput")
    cosk_d = nc.dram_tensor("cosk", [128, S], f32, kind="ExternalInput")
    sink_d = nc.dram_tensor("sink", [128, S], f32, kind="ExternalInput")
    bqk_d = nc.dram_tensor("bqk", [128, 16], f32, kind="ExternalInput")
    boutp_d = nc.dram_tensor("boutp", [128, 8], f32, kind="ExternalInput")
    out_d = nc.dram_tensor("outT", [8, 128, SQ], f32, kind="ExternalOutput")
    debug = bool(int(os.environ.get("KERNEL_DEBUG", "0") or "0"))
    if debug:
        dq_d = nc.dram_tensor("dq", [128, 8, 2, 512], bf16, kind="ExternalOutput")
        dk_d = nc.dram_tensor("dk", [128, 8, 4, 512], bf16, kind="ExternalOutput")
        dv_d = nc.dram_tensor("dv", [128, 16, NH, 72], bf16, kind="ExternalOutput")
        dctx_d = nc.dram_tensor("dctx", [128, KC, SQ], bf16, kind="ExternalOutput")

    with TileContext(nc) as tc:
        with (
            tc.tile_pool(name="const", bufs=1) as const,
            tc.tile_pool(name="persist", bufs=1) as persist,
            tc.tile_pool(name="xTp", bufs=1) as xTp,
            tc.tile_pool(name="mapp", bufs=1) as mapp,
            tc.tile_pool(name="wqkp", bufs=6) as wqkp,
            tc.tile_pool(name="ropep", bufs=2) as ropep,
            tc.tile_pool(name="ttcc", bufs=2) as ttcc,
            tc.tile_pool(name="qp", bufs=3) as qp,
            tc.tile_pool(name="kp", bufs=3) as kp,
            tc.tile_pool(name="woutp", bufs=1) as woutp,
            tc.tile_pool(name="ctxp", bufs=1) as ctxp,
            tc.tile_pool(name="expp", bufs=3) as expp,
            tc.tile_pool(name="scrp", bufs=2) as scrp,
            tc.tile_pool(name="obp", bufs=2) as obp,
        ):
            bqk_sb = const.tile([128, 16], f32)
            boutp_sb = const.tile([128, 8], f32)
            pswap_sb = const.tile([128, 128], bf16)

            # v resident in SBUF: [s2_in_blk, s2_blk, head, dcol+ones]
            v_sb = persist.tile([128, 16, NH, 72], bf16)
            ctxT = ctxp.tile([128, KC, SQ], bf16)
            cosk_sb = mapp.tile([128, 4, 512], f32)
            sink_sb = mapp.tile([128, 4, 512], f32)
            wout_sb = woutp.tile([128, 8, KC, 128], bf16, tag="wo", name="wout_sb")

            # ---------------- input DMA schedule -------------------------
            nc.gpsimd.memset(v_sb[:, :, :, DH : DH + 1], 1.0)
            nc.sync.dma_start(pswap_sb[:, :], pswap_d[:, :])
            xT_sb = [
                xTp.tile([128, S], bf16, tag=f"x{c}", name=f"xc{c}")
                for c in range(KC)
            ]
            wvp = tc.alloc_tile_pool(name="wvp", bufs=1)
            wvt = wvp.tile([128, KC, H], bf16, tag="wv", name="wvt")
            for c in range(KC):
                for h in range(4):
                    nc.sync.dma_start(
                        xT_sb[c][:, h * 512 : (h + 1) * 512],
                        xT_d[c, :, h * 512 : (h + 1) * 512],
                    )
                for h in range(2):
                    nc.sync.dma_start(
                        wvt[:, c, h * 512 : (h + 1) * 512],
                        wv_d[:, c, h * 512 : (h + 1) * 512],
                    )
            nc.sync.dma_start(bqk_sb[:, :], bqk_d[:, :])
            nc.sync.dma_start(boutp_sb[:, :], boutp_d[:, :])

            wt_tiles = {}

            def _wt_load(oc):
                wt = wqkp.tile([128, KC, 128], bf16, tag="w", name="wt")
                for h in range(2):
                    nc.sync.dma_start(
                        wt[:, h * 4 : (h + 1) * 4, :],
                        wqk_d[oc, :, h * 4 : (h + 1) * 4, :],
                    )
                wt_tiles[oc] = wt

            for p in range(2):
                _wt_load(p)
                _wt_load(8 + p)
            for h in range(4):
                nc.sync.dma_start(sink_sb[:, h, :], sink_d[:, h * 512 : (h + 1) * 512])
                nc.sync.dma_start(cosk_sb[:, h, :], cosk_d[:, h * 512 : (h + 1) * 512])
            for hb in range(8):
                nc.sync.dma_start(wout_sb[:, hb, :, :], wout_d[:, hb, :, :])

            # ---------------- phase B: v projection ----------------------
            with tc.tile_pool(name="psV", bufs=3, space="PSUM") as psV:
                for sb in range(16):
                    ps = psV.tile([128, H], f32, tag="psV", name="psV_t")
                    for c in range(KC):
                        for hv in range(2):
                            nc.tensor.matmul(
                                ps[:, hv * 512 : (hv + 1) * 512],
                                xT_sb[c][:, sb * 128 : (sb + 1) * 128],
                                wvt[:, c, hv * 512 : (hv + 1) * 512],
                                start=(c == 0), stop=(c == KC - 1),
                            )
                    nc.scalar.activation(
                        v_sb[:, sb, :, 0:DH],
                        ps.rearrange("p (h d) -> p h d", h=NH),
                        Copy,
                    )
            wvp.release()

            # ------------- merged qk-projection + attention loop ---------
            with (
                tc.tile_pool(name="scp", bufs=2, space="PSUM") as scp,
                tc.tile_pool(name="psCtx", bufs=2, space="PSUM") as psCtx,
            ):
                q_tiles, k_tiles = {}, {}
                pending = []

                def _stage2(state):
                    raw, tt, dst, sp = state
                    # sw = Pswap^T @ (raw*sin) ; dst = raw*cos + sw
                    swt = scp.tile([128, 2, 512], f32, tag="sc", name="sc")
                    for hv in range(2):
                        nc.tensor.matmul(
                            swt[:, hv, :], pswap_sb[:, :], tt[:, hv, :],
                            start=True, stop=True,
                        )
                    cc = ttcc.tile([128, 2, 512], f32, tag="cc", name="cc")
                    nc.gpsimd.tensor_tensor(
                        cc[:, :, :], raw[:, :, :],
                        cosk_sb[:, 2 * sp : 2 * sp + 2, :], MUL,
                    )
                    nc.vector.tensor_tensor(dst, cc[:, :, :], swt[:, :, :], ADD)

                def _qk_span(oc, sp, dst):
                    # dst: bf16 AP [128, 2, 512] getting rope(Wqk[oc]^T @ xT)
                    pst = scp.tile([128, 2, 512], f32, tag="sc", name="sc")
                    for c in range(KC):
                        for hv in range(2):
                            nc.tensor.matmul(
                                pst[:, hv, :],
                                wt_tiles[oc][:, c, :],
                                xT_sb[c][:, sp * SQ + hv * 512 : sp * SQ + (hv + 1) * 512],
                                start=(c == 0), stop=(c == KC - 1),
                            )
                    raw = ropep.tile([128, 2, 512], f32, tag="raw", name="raw")
                    nc.scalar.activation(
                        raw[:, :, :], pst[:, :, :], Ident,
                        bias=bqk_sb[:, oc : oc + 1],
                    )
                    tt = ttcc.tile([128, 2, 512], bf16, tag="tt", name="tt")
                    nc.vector.tensor_tensor(
                        tt[:, :, :], raw[:, :, :],
                        sink_sb[:, 2 * sp : 2 * sp + 2, :], MUL,
                    )
                    pending.append((raw, tt, dst, sp))
                    if len(pending) > 1:
                        _stage2(pending.pop(0))

                def _qk_pair_span(p, which):
                    # emit one of the 3 projection spans for pair p;
                    # prefetch pair p+2's weights only AFTER the last span
                    # so the 6-buf rotation never clobbers live tiles
                    if p > 7:
                        return
                    if which == 0:
                        q_tiles[p] = qp.tile([128, 2, 512], bf16, tag="q", name="qt")
                        _qk_span(p, 0, q_tiles[p][:, :, :])
                    elif which == 1:
                        k_tiles[p] = kp.tile([128, 4, 512], bf16, tag="k", name="kt")
                        _qk_span(8 + p, 0, k_tiles[p][:, 0:2, :])
                    else:
                        _qk_span(8 + p, 1, k_tiles[p][:, 2:4, :])
                        if p + 2 <= 7:
                            _wt_load(p + 2)
                            _wt_load(8 + p + 2)
                        if p == 7:
                            # last span: flush so pair 7's rope combine is
                            # emitted before its attention consumers
                            while pending:
                                _stage2(pending.pop(0))

                # prologue: q/k for pairs 0 and 1
                for p in range(2):
                    for w in range(3):
                        _qk_pair_span(p, w)

                for pr in range(8):
                    for hf in range(2):
                        s1 = slice(hf * 512, (hf + 1) * 512)
                        cE = psCtx.tile([128, 512], f32, tag="ctxe", name="cE")
                        cO = psCtx.tile([128, 512], f32, tag="ctxo", name="cO")
                        def _ctx(blk, et):
                            st, sp_ = (blk == 0), (blk == 15)
                            nc.tensor.matmul(
                                cE[0 : DH + 1, :],
                                v_sb[:, blk, 2 * pr, 0 : DH + 1], et[:, 0, :],
                                start=st, stop=sp_,
                            )
                            nc.tensor.matmul(
                                cO[0 : DH + 1, :],
                                v_sb[:, blk, 2 * pr + 1, 0 : DH + 1], et[:, 1, :],
                                start=st, stop=sp_,
                            )

                        # software-pipelined: ctx(blk-1) is emitted after
                        # scores(blk), giving exp(blk-1) a full block period
                        # before the PE needs its result
                        prev_et = None
                        for blk in range(16):
                            sc = scp.tile([128, 2, 512], f32, tag="sc", name="sc")
                            for par in range(2):
                                rs = par * 64
                                nc.tensor.matmul(
                                    sc[:, par, :],
                                    k_tiles[pr][
                                        rs : rs + 64, blk // 4,
                                        (blk % 4) * 128 : (blk % 4) * 128 + 128,
                                    ],
                                    q_tiles[pr][rs : rs + 64, hf, :],
                                    start=True, stop=True,
                                )
                            et = expp.tile([128, 2, 512], bf16, tag="et", name="et")
                            nc.scalar.activation(
                                et[:, :, :], sc[:, :, :], Exp, scale=0.125
                            )
                            if prev_et is not None:
                                _ctx(blk - 1, prev_et)
                            prev_et = et
                            # qk-projection for pair pr+2 rides inside
                            # hf=0's block stream
                            if hf == 0 and blk in (4, 9, 14):
                                _qk_pair_span(pr + 2, (4, 9, 14).index(blk))
                        _ctx(15, prev_et)
                        # epilogue: normalize ctx rows 0..63 by sums row 64.
                        # partition_broadcast reads tensor partition 0, so
                        # route the sums row through partition 0 via DMA.
                        for par, ct in ((0, cE), (1, cO)):
                            scr = scrp.tile([128, 512], f32, tag="scr", name="scr")
                            nc.vector.tensor_copy(scr[64:65, :], ct[64:65, :])
                            scr2 = scrp.tile([1, 512], f32, tag="scr2", name="scr2")
                            nc.sync.dma_start(scr2[0:1, :], scr[64:65, :])
                            bcs = scrp.tile([128, 512], f32, tag="bcs", name="bcs")
                            nc.gpsimd.partition_broadcast(bcs[0:64, :], scr2[0:1, :])
                            bc = bcs
                            nc.vector.reciprocal_approx_fast(bc[0:64, :], bcs[0:64, :])
                            if par == 0:
                                nc.vector.tensor_tensor(
                                    ctxT[0:64, pr, s1], ct[0:64, :], bc[0:64, :], MUL
                                )
                            else:
                                tmp = scrp.tile([64, 512], bf16, tag="tmp", name="tmp")
                                nc.vector.tensor_tensor(
                                    tmp[:, :], ct[0:64, :], bc[0:64, :], MUL
                                )
                                nc.sync.dma_start(ctxT[64:128, pr, s1], tmp[:, :])
                while pending:
                    _stage2(pending.pop(0))

                # ---------------- phase D: out projection ----------------
                for hf in range(2):
                    s1 = slice(hf * 512, (hf + 1) * 512)
                    for hb in range(8):
                        pst = scp.tile([128, 2, 512], f32, tag="sc", name="sc")
                        ps = pst[:, 0, :]
                        for c in range(KC):
                            nc.tensor.matmul(
                                ps,
                                wout_sb[:, hb, c, :],
                                ctxT[:, c, s1],
                                start=(c == 0), stop=(c == KC - 1),
                            )
                        ob = obp.tile([128, 512], f32, tag="ob", name="ob")
                        nc.scalar.activation(
                            ob[:, :], ps, Ident, bias=boutp_sb[:, hb : hb + 1]
                        )
                        for h in range(2):
                            nc.sync.dma_start(
                                out_d[hb, :, hf * 512 + h * 256 : hf * 512 + (h + 1) * 256],
                                ob[:, h * 256 : (h + 1) * 256],
                            )

            if debug:
                for p in range(8):
                    nc.sync.dma_start(dq_d[:, p, :, :], q_tiles[p][:, :, :])
                    nc.sync.dma_start(dk_d[:, p, :, :], k_tiles[p][:, :, :])
                nc.sync.dma_start(dv_d[:, :, :, :], v_sb[:, :, :, :])
                nc.sync.dma_start(dctx_d[:, :, :], ctxT[:, :, :])

    nc.finalize()
    return nc


def _host_prep(positions, hidden_states, Wqkv, bqkv, Wout, bout):
    import ml_dtypes

    bf16 = ml_dtypes.bfloat16
    positions = np.asarray(positions)
    hidden_states = np.asarray(hidden_states, dtype=np.float32)
    Wqkv = np.asarray(Wqkv, dtype=np.float32)
    bqkv = np.asarray(bqkv, dtype=np.float32)
    Wout = np.asarray(Wout, dtype=np.float32)
    bout = np.asarray(bout, dtype=np.float32)

    # wqk[oc][p][c][128]: per-oc weight tile with 2KB-contiguous lines
    wqk = np.ascontiguousarray(
        Wqkv[:, : 2 * H].reshape(KC, 128, 16, 128).transpose(2, 1, 0, 3)
    ).astype(bf16)
    # wv[p][c][H]: single-tile load, partition = row within chunk
    wv = np.ascontiguousarray(
        Wqkv[:, 2 * H :].reshape(KC, 128, H).transpose(1, 0, 2)
    ).astype(bf16)
    # wout[p][hb][c][128]
    wout_t = np.ascontiguousarray(
        Wout.reshape(KC, 128, 8, 128).transpose(1, 2, 0, 3)
    ).astype(bf16)
    bqk = np.ascontiguousarray(bqkv[: 2 * H].reshape(16, 128).T)
    boutp_full = bout.astype(np.float64) + bqkv[2 * H :].astype(
        np.float64
    ) @ Wout.astype(np.float64)
    boutp = np.ascontiguousarray(boutp_full.astype(np.float32).reshape(8, 128).T)

    pswap = np.zeros((128, 128), dtype=np.float32)
    for m in range(128):
        if m % DH < HALF:
            pswap[m + HALF, m] = -1.0
        else:
            pswap[m - HALF, m] = 1.0
    pswap = pswap.astype(bf16)

    inv_freq = 1.0 / (ROPE_BASE ** (np.arange(HALF, dtype=np.float64) / HALF))
    rowmap = np.arange(128) % HALF

    in_maps = []
    for c in range(N_CORES):
        b, hf = c // 2, c % 2
        perm = np.concatenate(
            [np.arange(hf * SQ, (hf + 1) * SQ), np.arange((1 - hf) * SQ, (2 - hf) * SQ)]
        )
        x_perm = hidden_states[b][perm]
        xT = np.ascontiguousarray(x_perm.T).reshape(KC, 128, S).astype(bf16)
        pos = positions[perm].astype(np.float64)
        freqs = pos[:, None] * inv_freq[None, :]  # [S, HALF]
        cosk = np.ascontiguousarray(np.cos(freqs).astype(np.float32)[:, rowmap].T)
        sink = np.ascontiguousarray(np.sin(freqs).astype(np.float32)[:, rowmap].T)
        in_maps.append(
            {
                "xT": xT, "wqk": wqk, "wv": wv, "wout": wout_t,
                "pswap": pswap, "cosk": cosk, "sink": sink,
                "bqk": bqk, "boutp": boutp,
            }
        )
    return in_maps


def kernel(positions, hidden_states, Wqkv, bqkv, Wout, bout):
    global _nc_cache, last_results
    from concourse import bass_utils

    if _nc_cache is None:
        _nc_cache = _build_nc()
    nc = _nc_cache

    in_maps = _host_prep(positions, hidden_states, Wqkv, bqkv, Wout, bout)
    res = bass_utils.run_bass_kernel_spmd(
        nc, in_maps, core_ids=list(range(N_CORES)),
        trace=bool(int(os.environ.get("KERNEL_TRACE", "0") or "0")),
    )
    last_results = res

    out = np.empty((B, S, H), dtype=np.float32)
    for c in range(N_CORES):
        b, hf = c // 2, c % 2
        outT = np.asarray(res.results[c]["outT"]).reshape(H, SQ)
        out[b, hf * SQ : (hf + 1) * SQ, :] = outT.T
    return out


# revision 60
# speedup vs baseline: 1.0095x; 1.0095x over previous
"""BERT-with-RoPE attention layer on 8 Trainium2 NeuronCores.

Sharding: core c handles (batch b = c//2, sequence-half hf = c%2).
Each core computes k/v for its batch's full 2048 rows (k/v work duplicated
2x across the pair) and q + attention + out-projection for its own 1024
query rows, so the 8 output shards are disjoint and the host gather is a
pure concatenation (no collectives).

v6 — pair-pipelined: the q/k projection (+RoPE) for pair p+2 is emitted
inside pair p's attention block stream, so the PE never idles while the
scalar engine grinds through the exp stream. Phase order on device:
  B:  v = xT^T @ Wv (SBUF-resident, ones column via memset)
  loop over pairs p (pair-major, hf inner):
      attention(p, hf=0/1) with qk-projection spans for pair p+2
      interleaved between score/ctx blocks; softmax via ones-column sums,
      normalize epilogue off the PE critical path
  D:  outT = Wout^T-slices @ ctxT (weights preloaded, PSUM borrowed from
      the scores pool; runs under the tail of the exp stream)
All q/k/sw/scores/D PSUM accumulators share one [128,2,512] tile pool
(6 banks) + 2 banks for the ctx accumulators = exactly 8 PSUM banks.
"""

import os
import numpy as np

B, S, H = 4, 2048, 1024
NH, DH = 16, 64
HALF = DH // 2
SQ = S // 2  # query rows per core
KC = H // 128  # hidden contraction chunks
ROPE_BASE = 10000.0
N_CORES = 8

_nc_cache = None
last_results = None


def _build_nc():
    import concourse.bacc as bacc
    import concourse.mybir as mybir
    from concourse.tile import TileContext

    f32 = mybir.dt.float32
    bf16 = mybir.dt.bfloat16
    Exp = mybir.ActivationFunctionType.Exp
    Ident = mybir.ActivationFunctionType.Identity
    Copy = mybir.ActivationFunctionType.Copy
    MUL = mybir.AluOpType.mult
    ADD = mybir.AluOpType.add

    nc = bacc.Bacc(None, target_bir_lowering=False)

    xT_d = nc.dram_tensor("xT", [KC, 128, S], bf16, kind="ExternalInput")
    pswap_d = nc.dram_tensor("pswap", [128, 128], bf16, kind="ExternalInput")
    wqk_d = nc.dram_tensor("wqk", [16, 128, KC, 128], bf16, kind="ExternalInput")
    wv_d = nc.dram_tensor("wv", [128, KC, H], bf16, kind="ExternalInput")
    wout_d = nc.dram_tensor("wout", [128, 8, KC, 128], bf16, kind="ExternalInput")
    cosk_d = nc.dram_tensor("cosk", [128, S], f32, kind="ExternalInput")
    sink_d = nc.dram_tensor("sink", [128, S], f32, kind="ExternalInput")
    bqk_d = nc.dram_tensor("bqk", [128, 16], f32, kind="ExternalInput")
    boutp_d = nc.dram_tensor("boutp", [128, 8], f32, kind="ExternalInput")
    out_d = nc.dram_tensor("outT", [8, 128, SQ], f32, kind="ExternalOutput")
    debug = bool(int(os.environ.get("KERNEL_DEBUG", "0") or "0"))
    if debug:
        dq_d = nc.dram_tensor("dq", [128, 8, 2, 512], bf16, kind="ExternalOutput")
        dk_d = nc.dram_tensor("dk", [128, 8, 4, 512], bf16, kind="ExternalOutput")
        dv_d = nc.dram_tensor("dv", [128, 16, NH, 72], bf16, kind="ExternalOutput")
        dctx_d = nc.dram_tensor("dctx", [128, KC, SQ], bf16, kind="ExternalOutput")

    with TileContext(nc) as tc:
        with (
            tc.tile_pool(name="const", bufs=1) as const,
            tc.tile_pool(name="persist", bufs=1) as persist,
            tc.tile_pool(name="xTp", bufs=1) as xTp,
            tc.tile_pool(name="mapp", bufs=1) as mapp,
            tc.tile_pool(name="wqkp", bufs=6) as wqkp,
            tc.tile_pool(name="ropep", bufs=2) as ropep,
            tc.tile_pool(name="ttcc", bufs=2) as ttcc,
            tc.tile_pool(name="qp", bufs=3) as qp,
            tc.tile_pool(name="kp", bufs=3) as kp,
            tc.tile_pool(name="woutp", bufs=1) as woutp,
            tc.tile_pool(name="ctxp", bufs=1) as ctxp,
            tc.tile_pool(name="expp", bufs=2) as expp,
            tc.tile_pool(name="scrp", bufs=2) as scrp,
            tc.tile_pool(name="obp", bufs=2) as obp,
        ):
            bqk_sb = const.tile([128, 16], f32)
            boutp_sb = const.tile([128, 8], f32)
            pswap_sb = const.tile([128, 128], bf16)

            # v resident in SBUF: [s2_in_blk, s2_blk, head, dcol+ones]
            v_sb = persist.tile([128, 16, NH, 72], bf16)
            ctxT = ctxp.tile([128, KC, SQ], bf16)
            cosk_sb = mapp.tile([128, 4, 512], f32)
            sink_sb = mapp.tile([128, 4, 512], f32)
            wout_sb = woutp.tile([128, 8, KC, 128], bf16, tag="wo", name="wout_sb")

            # ---------------- input DMA schedule -------------------------
            nc.gpsimd.memset(v_sb[:, :, :, DH : DH + 1], 1.0)
            nc.sync.dma_start(pswap_sb[:, :], pswap_d[:, :])
            xT_sb = [
                xTp.tile([128, S], bf16, tag=f"x{c}", name=f"xc{c}")
                for c in range(KC)
            ]
            wvp = tc.alloc_tile_pool(name="wvp", bufs=1)
            wvt = wvp.tile([128, KC, H], bf16, tag="wv", name="wvt")
            for c in range(KC):
                for h in range(4):
                    nc.sync.dma_start(
                        xT_sb[c][:, h * 512 : (h + 1) * 512],
                        xT_d[c, :, h * 512 : (h + 1) * 512],
                    )
                nc.sync.dma_start(wvt[:, c, :], wv_d[:, c, :])
            nc.sync.dma_start(bqk_sb[:, :], bqk_d[:, :])
            nc.sync.dma_start(boutp_sb[:, :], boutp_d[:, :])

            wt_tiles = {}

            def _wt_load(oc):
                wt = wqkp.tile([128, KC, 128], bf16, tag="w", name="wt")
                for h in range(2):
                    nc.sync.dma_start(
                        wt[:, h * 4 : (h + 1) * 4, :],
                        wqk_d[oc, :, h * 4 : (h + 1) * 4, :],
                    )
                wt_tiles[oc] = wt

            for p in range(2):
                _wt_load(p)
                _wt_load(8 + p)
            for h in range(4):
                nc.sync.dma_start(sink_sb[:, h, :], sink_d[:, h * 512 : (h + 1) * 512])
                nc.sync.dma_start(cosk_sb[:, h, :], cosk_d[:, h * 512 : (h + 1) * 512])
            for hb in range(8):
                nc.sync.dma_start(wout_sb[:, hb, :, :], wout_d[:, hb, :, :])

            # ---------------- phase B: v projection ----------------------
            with tc.tile_pool(name="psV", bufs=3, space="PSUM") as psV:
                for sb in range(16):
                    ps = psV.tile([128, H], f32, tag="psV", name="psV_t")
                    for c in range(KC):
                        for hv in range(2):
                            nc.tensor.matmul(
                                ps[:, hv * 512 : (hv + 1) * 512],
                                xT_sb[c][:, sb * 128 : (sb + 1) * 128],
                                wvt[:, c, hv * 512 : (hv + 1) * 512],
                                start=(c == 0), stop=(c == KC - 1),
                            )
                    nc.scalar.activation(
                        v_sb[:, sb, :, 0:DH],
                        ps.rearrange("p (h d) -> p h d", h=NH),
                        Copy,
                    )
            wvp.release()

            # ------------- merged qk-projection + attention loop ---------
            with (
                tc.tile_pool(name="scp", bufs=2, space="PSUM") as scp,
                tc.tile_pool(name="psCtx", bufs=2, space="PSUM") as psCtx,
            ):
                q_tiles, k_tiles = {}, {}
                pending = []

                def _stage2(state):
                    raw, tt, dst, sp = state
                    # sw = Pswap^T @ (raw*sin) ; dst = raw*cos + sw
                    swt = scp.tile([128, 2, 512], f32, tag="sc", name="sc")
                    for hv in range(2):
                        nc.tensor.matmul(
                            swt[:, hv, :], pswap_sb[:, :], tt[:, hv, :],
                            start=True, stop=True,
                        )
                    cc = ttcc.tile([128, 2, 512], f32, tag="cc", name="cc")
                    nc.gpsimd.tensor_tensor(
                        cc[:, :, :], raw[:, :, :],
                        cosk_sb[:, 2 * sp : 2 * sp + 2, :], MUL,
                    )
                    nc.vector.tensor_tensor(dst, cc[:, :, :], swt[:, :, :], ADD)

                def _qk_span(oc, sp, dst):
                    # dst: bf16 AP [128, 2, 512] getting rope(Wqk[oc]^T @ xT)
                    pst = scp.tile([128, 2, 512], f32, tag="sc", name="sc")
                    for c in range(KC):
                        for hv in range(2):
                            nc.tensor.matmul(
                                pst[:, hv, :],
                                wt_tiles[oc][:, c, :],
                                xT_sb[c][:, sp * SQ + hv * 512 : sp * SQ + (hv + 1) * 512],
                                start=(c == 0), stop=(c == KC - 1),
                            )
                    raw = ropep.tile([128, 2, 512], f32, tag="raw", name="raw")
                    nc.scalar.activation(
                        raw[:, :, :], pst[:, :, :], Ident,
                        bias=bqk_sb[:, oc : oc + 1],
                    )
                    tt = ttcc.tile([128, 2, 512], bf16, tag="tt", name="tt")
                    nc.vector.tensor_tensor(
                        tt[:, :, :], raw[:, :, :],
                        sink_sb[:, 2 * sp : 2 * sp + 2, :], MUL,
                    )
                    pending.append((raw, tt, dst, sp))
                    if len(pending) > 1:
                        _stage2(pending.pop(0))

                def _qk_pair_span(p, which):
                    # emit one of the 3 projection spans for pair p;
                    # prefetch pair p+2's weights only AFTER the last span
                    # so the 6-buf rotation never clobbers live tiles
                    if p > 7:
                        return
                    if which == 0:
                        q_tiles[p] = qp.tile([128, 2, 512], bf16, tag="q", name="qt")
                        _qk_span(p, 0, q_tiles[p][:, :, :])
                    elif which == 1:
                        k_tiles[p] = kp.tile([128, 4, 512], bf16, tag="k", name="kt")
                        _qk_span(8 + p, 0, k_tiles[p][:, 0:2, :])
                    else:
                        _qk_span(8 + p, 1, k_tiles[p][:, 2:4, :])
                        if p + 2 <= 7:
                            _wt_load(p + 2)
                            _wt_load(8 + p + 2)
                        if p == 7:
                            # last span: flush so pair 7's rope combine is
                            # emitted before its attention consumers
                            while pending:
                                _stage2(pending.pop(0))

                # prologue: q/k for pairs 0 and 1
                for p in range(2):
                    for w in range(3):
                        _qk_pair_span(p, w)

                for pr in range(8):
                    for hf in range(2):
                        s1 = slice(hf * 512, (hf + 1) * 512)
                        cE = psCtx.tile([128, 512], f32, tag="ctxe", name="cE")
                        cO = psCtx.tile([128, 512], f32, tag="ctxo", name="cO")
                        def _ctx(blk, et):
                            st, sp_ = (blk == 0), (blk == 15)
                            nc.tensor.matmul(
                                cE[0 : DH + 1, :],
                                v_sb[:, blk, 2 * pr, 0 : DH + 1], et[:, 0, :],
                                start=st, stop=sp_,
                            )
                            nc.tensor.matmul(
                                cO[0 : DH + 1, :],
                                v_sb[:, blk, 2 * pr + 1, 0 : DH + 1], et[:, 1, :],
                                start=st, stop=sp_,
                            )

                        # software-pipelined: ctx(blk-1) is emitted after
                        # scores(blk), giving exp(blk-1) a full block period
                        # before the PE needs its result
                        prev_et = None
                        for blk in range(16):
                            sc = scp.tile([128, 2, 512], f32, tag="sc", name="sc")
                            for par in range(2):
                                rs = par * 64
                                nc.tensor.matmul(
                                    sc[:, par, :],
                                    k_tiles[pr][
                                        rs : rs + 64, blk // 4,
                                        (blk % 4) * 128 : (blk % 4) * 128 + 128,
                                    ],
                                    q_tiles[pr][rs : rs + 64, hf, :],
                                    start=True, stop=True,
                                )
                            et = expp.tile([128, 2, 512], bf16, tag="et", name="et")
                            nc.scalar.activation(
                                et[:, :, :], sc[:, :, :], Exp, scale=0.125
                            )
                            if prev_et is not None:
                                _ctx(blk - 1, prev_et)
                            prev_et = et
                            # qk-projection for pair pr+2 rides inside
                            # hf=0's block stream
                            if hf == 0 and blk in (4, 9, 14):
                                _qk_pair_span(pr + 2, (4, 9, 14).index(blk))
                        _ctx(15, prev_et)
                        # epilogue: normalize ctx rows 0..63 by sums row 64.
                        # partition_broadcast reads tensor partition 0, so
                        # route the sums row through partition 0 via DMA.
                        for par, ct in ((0, cE), (1, cO)):
                            scr = scrp.tile([128, 512], f32, tag="scr", name="scr")
                            nc.vector.tensor_copy(scr[64:65, :], ct[64:65, :])
                            scr2 = scrp.tile([1, 512], f32, tag="scr2", name="scr2")
                            nc.sync.dma_start(scr2[0:1, :], scr[64:65, :])
                            bcs = scrp.tile([128, 512], f32, tag="bcs", name="bcs")
                            nc.gpsimd.partition_broadcast(bcs[0:64, :], scr2[0:1, :])
                            bc = bcs
                            nc.vector.reciprocal_approx_fast(bc[0:64, :], bcs[0:64, :])
                            if par == 0:
                                nc.vector.tensor_tensor(
                                    ctxT[0:64, pr, s1], ct[0:64, :], bc[0:64, :], MUL
                                )
                            else:
                                tmp = scrp.tile([64, 512], bf16, tag="tmp", name="tmp")
                                nc.vector.tensor_tensor(
                                    tmp[:, :], ct[0:64, :], bc[0:64, :], MUL
                                )
                                nc.sync.dma_start(ctxT[64:128, pr, s1], tmp[:, :])
                while pending:
                    _stage2(pending.pop(0))

                # ---------------- phase D: out projection ----------------
                for hf in range(2):
                    s1 = slice(hf * 512, (hf + 1) * 512)
                    for hb in range(8):
                        pst = scp.tile([128, 2, 512], f32, tag="sc", name="sc")
                        ps = pst[:, 0, :]
                        for c in range(KC):
                            nc.tensor.matmul(
                                ps,
                                wout_sb[:, hb, c, :],
                                ctxT[:, c, s1],
                                start=(c == 0), stop=(c == KC - 1),
                            )
                        ob = obp.tile([128, 512], f32, tag="ob", name="ob")
                        nc.scalar.activation(
                            ob[:, :], ps, Ident, bias=boutp_sb[:, hb : hb + 1]
                        )
                        for h in range(2):
                            nc.sync.dma_start(
                                out_d[hb, :, hf * 512 + h * 256 : hf * 512 + (h + 1) * 256],
                                ob[:, h * 256 : (h + 1) * 256],
                            )

            if debug:
                for p in range(8):
                    nc.sync.dma_start(dq_d[:, p, :, :], q_tiles[p][:, :, :])
                    nc.sync.dma_start(dk_d[:, p, :, :], k_tiles[p][:, :, :])
                nc.sync.dma_start(dv_d[:, :, :, :], v_sb[:, :, :, :])
                nc.sync.dma_start(dctx_d[:, :, :], ctxT[:, :, :])

    nc.finalize()
    return nc


def _host_prep(positions, hidden_states, Wqkv, bqkv, Wout, bout):
    import ml_dtypes

    bf16 = ml_dtypes.bfloat16
    positions = np.asarray(positions)
    hidden_states = np.asarray(hidden_states, dtype=np.float32)
    Wqkv = np.asarray(Wqkv, dtype=np.float32)
    bqkv = np.asarray(bqkv, dtype=np.float32)
    Wout = np.asarray(Wout, dtype=np.float32)
    bout = np.asarray(bout, dtype=np.float32)

    # wqk[oc][p][c][128]: per-oc weight tile with 2KB-contiguous lines
    wqk = np.ascontiguousarray(
        Wqkv[:, : 2 * H].reshape(KC, 128, 16, 128).transpose(2, 1, 0, 3)
    ).astype(bf16)
    # wv[p][c][H]: single-tile load, partition = row within chunk
    wv = np.ascontiguousarray(
        Wqkv[:, 2 * H :].reshape(KC, 128, H).transpose(1, 0, 2)
    ).astype(bf16)
    # wout[p][hb][c][128]
    wout_t = np.ascontiguousarray(
        Wout.reshape(KC, 128, 8, 128).transpose(1, 2, 0, 3)
    ).astype(bf16)
    bqk = np.ascontiguousarray(bqkv[: 2 * H].reshape(16, 128).T)
    boutp_full = bout.astype(np.float64) + bqkv[2 * H :].astype(
        np.float64
    ) @ Wout.astype(np.float64)
    boutp = np.ascontiguousarray(boutp_full.astype(np.float32).reshape(8, 128).T)

    pswap = np.zeros((128, 128), dtype=np.float32)
    for m in range(128):
        if m % DH < HALF:
            pswap[m + HALF, m] = -1.0
        else:
            pswap[m - HALF, m] = 1.0
    pswap = pswap.astype(bf16)

    inv_freq = 1.0 / (ROPE_BASE ** (np.arange(HALF, dtype=np.float64) / HALF))
    rowmap = np.arange(128) % HALF

    in_maps = []
    for c in range(N_CORES):
        b, hf = c // 2, c % 2
        perm = np.concatenate(
            [np.arange(hf * SQ, (hf + 1) * SQ), np.arange((1 - hf) * SQ, (2 - hf) * SQ)]
        )
        x_perm = hidden_states[b][perm]
        xT = np.ascontiguousarray(x_perm.T).reshape(KC, 128, S).astype(bf16)
        pos = positions[perm].astype(np.float64)
        freqs = pos[:, None] * inv_freq[None, :]  # [S, HALF]
        cosk = np.ascontiguousarray(np.cos(freqs).astype(np.float32)[:, rowmap].T)
        sink = np.ascontiguousarray(np.sin(freqs).astype(np.float32)[:, rowmap].T)
        in_maps.append(
            {
                "xT": xT, "wqk": wqk, "wv": wv, "wout": wout_t,
                "pswap": pswap, "cosk": cosk, "sink": sink,
                "bqk": bqk, "boutp": boutp,
            }
        )
    return in_maps


def kernel(positions, hidden_states, Wqkv, bqkv, Wout, bout):
    global _nc_cache, last_results
    from concourse import bass_utils

    if _nc_cache is None:
        _nc_cache = _build_nc()
    nc = _nc_cache

    in_maps = _host_prep(positions, hidden_states, Wqkv, bqkv, Wout, bout)
    res = bass_utils.run_bass_kernel_spmd(
        nc, in_maps, core_ids=list(range(N_CORES)),
        trace=bool(int(os.environ.get("KERNEL_TRACE", "0") or "0")),
    )
    last_results = res

    out = np.empty((B, S, H), dtype=np.float32)
    for c in range(N_CORES):
        b, hf = c // 2, c % 2
        outT = np.asarray(res.results[c]["outT"]).reshape(H, SQ)
        out[b, hf * SQ : (hf + 1) * SQ, :] = outT.T
    return out
